# revision 43
# baseline (speedup 1.0000x reference)
"""MEGADecoder forward pass as a Bass/Tile kernel on 8 TRN2 NeuronCores.

Sharding: sequence-parallel. Each core owns SEQ/8 = 512 rows. Params are
replicated. Single-head full attention uses one AllGather of (K.T, V).

Layout: activations are stored feature-major ([8 chunks x 128 partitions,
seq 512 free]) so every GEMM is a chain of PE matmuls with no transposes:
  - projections:  out.T[o, s] = sum_d W.T[d, o] . act.T[d, s]
  - V projection: V[s, o]     = sum_d Z.T[d, s] . Wv.T[d, o]   (seq-major out)
  - scores.T:     S.T[j, i]   = sum_f K.T[f, j] . Q.T[f, i]
  - attention:    Zat.T[o, s] = sum_j V[j, o]   . P.T[j, s]
Softmax runs without max-subtraction (scores are O(1)); the denominator
comes from a ones-vector matmul, its reciprocal is broadcast across
partitions with a rank-1 matmul (no DRAM bounce), and 1/l is folded into
the attention gate f during 5B.

fp8 (e4m3) + DoubleRow: every GEMM except phase-1 (alpha/delta), W_EMA and
the tiny heads runs with both operands in fp8e4 and perf_mode=DoubleRow
(256-deep contraction per instruction, ~1.4x the bf16 matmul rate).
Numerics were validated against a numpy bit-model of this exact pipeline:
bf16 everywhere = 7.86e-3 max-rel error, this fp8 split = 8.2e-3 (tolerance
2e-2); putting phase-1 or W_EMA in fp8 blows up to 6e-2, so those stay bf16.
Scales: weights x32 (their 0.054 max would otherwise sit in subnormals),
V x16, pt = 8*exp(s) via an ln(8) bias folded into the Exp activation;
activations are stored unscaled (std ~0.5-1.2 is mid-range for e4m3).
All descales fold into existing activation/DVE drains. TRN fp8 e4m3
overflows to Inf above 240 (no saturation): measured maxes are pt 120,
V*16 64, zatp*16 12, weights*32 1.73.

The K.T / V AllGather payloads are fp8 (0.5 MB in, 4 MB out per tensor),
and phase 5 re-reads 8 MB instead of 16 MB. Weight loads are issued early
inside the phase-1 loop (wz/wk at ot==1, wv/wq at ot==3, wf at ot==5,
wema at ot==6) so no GEMM waits on its weights.
"""

import numpy as np

SEQ = 4096
D = 1024
NCORES = 8
S = SEQ // NCORES  # 512 rows per core
P = 128
FC = D // P  # 8 feature chunks
ATT_SCALE = 1.0 / float(np.sqrt(np.float32(D)))
S_W = 32.0   # fp8 weight scale
S_V = 16.0   # fp8 V scale
S_P = 8.0    # pt = S_P * exp(score)
LN_SP = float(np.log(S_P))

_CACHE = {}


def _build_bass(reps=1, nocoll=False):
    import concourse.bacc as bacc
    import concourse.tile as tile
    import concourse.mybir as mybir

    f32 = mybir.dt.float32
    bf16 = mybir.dt.bfloat16
    fp8 = mybir.dt.float8e4
    AF = mybir.ActivationFunctionType

    nc = bacc.Bacc(None, target_bir_lowering=False, num_devices=NCORES)
    mm = nc.tensor.matmul

    # ---- DRAM I/O (all host-packed layouts) ----
    rt = nc.dram_tensor("rt", [P, FC, S + 1], bf16, kind="ExternalInput")
    # phase-1 weights: [ot, p, k(2FC), o(P)]
    wa = nc.dram_tensor("wa", [FC, P, 2 * FC, P], bf16, kind="ExternalInput")
    wd = nc.dram_tensor("wd", [FC, P, 2 * FC, P], bf16, kind="ExternalInput")
    # fp8 projection weights (x32): [half, p, c(FC), o(4P)]
    w_in = {}
    for name in ["wz", "wq", "wk", "wv", "wf", "wzat"]:
        w_in[name] = nc.dram_tensor(name, [2, P, FC, 4 * P], fp8,
                                    kind="ExternalInput")
    wema = nc.dram_tensor("wema", [2, P, FC, 4 * P], bf16,
                          kind="ExternalInput")
    wi = nc.dram_tensor("wi", [P, FC], bf16, kind="ExternalInput")
    wfin = nc.dram_tensor("wfin", [P, FC], bf16, kind="ExternalInput")
    # biases packed [p, 10, FC]: rows alpha,delta,z,q,k,v,ema,f,zat,i
    biases = nc.dram_tensor("biases", [P, 10, FC], f32, kind="ExternalInput")
    bvrow = nc.dram_tensor("bvrow", [1, D], f32, kind="ExternalInput")
    out = nc.dram_tensor("out", [S, 1], f32, kind="ExternalOutput")

    KT_ELEMS = D * S
    V_ELEMS = S * D
    NJ = SEQ // P  # 32 j-chunks

    with tile.TileContext(nc) as tc, \
         tc.tile_pool(name="consts", bufs=1) as consts, \
         tc.tile_pool(name="dram", bufs=1, space="DRAM") as dram, \
         tc.tile_pool(name="big", bufs=1) as big, \
         tc.tile_pool(name="pw8", bufs=1) as pw8, \
         tc.tile_pool(name="p_rt", bufs=2) as p_rt, \
         tc.tile_pool(name="p1w", bufs=2) as p1w, \
         tc.tile_pool(name="gps", bufs=1, space="PSUM") as gps:

        bsb = consts.tile([P, 10, FC], f32)
        nc.sync.dma_start(out=bsb, in_=biases.ap())
        ones_dr = consts.tile([P, 2, 16], fp8)
        nc.vector.memset(ones_dr, 2.0)  # folds the 0.5 of sigmoid-via-tanh
        lnsp_col = consts.tile([P, 1], f32)
        nc.vector.memset(lnsp_col, LN_SP)
        bv_b = consts.tile([P, D], f32)
        nc.sync.dma_start(out=bv_b, in_=bvrow.ap().partition_broadcast(P))

        def bias_ap(row, chunk):
            return bsb[:, row, chunk:chunk + 1]

        # persistent weight tiles: hoisted out of the rep loop so rep i+1's
        # reloads only WAR rep i's last reader of the same tile (mid-rep),
        # instead of hitting a pool-alloc barrier at rep end.
        w8 = {}
        for name in ["wz", "wk", "wv", "wq", "wf", "wzat"]:
            w8[name] = [pw8.tile([P, FC, 4 * P], fp8, name=f"{name}{h}")
                        for h in range(2)]
        wema_sb = [pw8.tile([P, FC, 4 * P], bf16, name=f"wema{h}")
                   for h in range(2)]

        # the 8 physical PSUM banks as global tiles, shared by every
        # phase and rep: subtile WAR/RAW deps replace pool-alloc barriers,
        # so e.g. rep i+1's phase-1 chains start as soon as rep i's head
        # drains the individual bank, not when the whole head finishes
        bank = [gps.tile([P, S], f32, name=f"bank{i}") for i in range(FC)]

        # whole-kernel resident activations
        rema = big.tile([P, FC, S], bf16, name="rema")
        rema8 = big.tile([P, FC, S], fp8, name="rema8")
        z8 = big.tile([P, FC, S], fp8, name="z8")
        qT8 = big.tile([P, FC, S], fp8, name="qT8")
        remap = big.tile([P, FC, S], bf16, name="remap")
        remap8 = big.tile([P, FC, S], fp8, name="remap8")
        fT = big.tile([P, FC, S], bf16, name="fT")
        zatp8 = big.tile([P, FC, S], fp8, name="zatp8")
        i_row = big.tile([1, S], f32, name="i_row")
        fin_rem = big.tile([1, S], f32, name="fin_rem")

        # next-rep input prefetch: rep i+1's rt / first wa+wd loads are
        # emitted mid-way through rep i's phase 5, so the next iteration's
        # phase 1 starts with its inputs already resident instead of
        # queueing its DMAs behind rep i's entire stream.
        handoff = {}

        def make_prefetch(rep_idx):
            def cb():
                if rep_idx + 1 >= reps:
                    return
                nrt = p_rt.tile([P, FC, S + 1], bf16, tag="rt",
                                name=f"rt{rep_idx + 1}")
                nc.sync.dma_start(out=nrt[:, 0:FC // 2, :],
                                  in_=rt.ap()[:, 0:FC // 2, :])
                nc.sync.dma_start(out=nrt[:, FC // 2:, :],
                                  in_=rt.ap()[:, FC // 2:, :])
                nwa = p1w.tile([P, 2 * FC, P], bf16, tag="wa",
                               name=f"wa0_{rep_idx + 1}")
                nc.sync.dma_start(out=nwa, in_=wa.ap()[0])
                nwd = p1w.tile([P, 2 * FC, P], bf16, tag="wd",
                               name=f"wd0_{rep_idx + 1}")
                nc.sync.dma_start(out=nwd, in_=wd.ap()[0])
                handoff["rt"] = nrt
                handoff["wa0"] = nwa
                handoff["wd0"] = nwd
            return cb

        def emit_p1_for(rep_idx):
            def cb():
                if rep_idx >= reps:
                    return
                _emit_p1(nc, tc, mybir, AF, bf16, f32, fp8, mm, rt, wa, wd,
                         w_in, wema, bsb, bias_ap, rema, rema8, w8, wema_sb,
                         p_rt, p1w, handoff, bank)
            return cb

        for _rep in range(reps):
            emit_p1_for(_rep)()
            row_bounce = dram.tile([1, S], f32, name=f"rb{_rep}")
            kt_in = dram.tile([KT_ELEMS], fp8, name=f"kti{_rep}")
            v_in = dram.tile([V_ELEMS], fp8, name=f"vi{_rep}")
            if nocoll:
                kt_out = v_out = None
            else:
                kt_out = dram.tile([NCORES, KT_ELEMS], fp8,
                                   addr_space="Shared", name=f"kto{_rep}")
                v_out = dram.tile([NCORES, V_ELEMS], fp8,
                                  addr_space="Shared", name=f"vo{_rep}")
            _emit_rest(nc, tc, mybir, AF, bf16, f32, fp8, mm, rt, wa, wd,
                       w_in, wema, wi, wfin, out, row_bounce, kt_in, kt_out,
                       v_in, v_out, bsb, bias_ap, ones_dr, lnsp_col, bv_b,
                       rema, rema8, z8, qT8, remap, remap8, fT, zatp8, i_row,
                       fin_rem, KT_ELEMS, V_ELEMS, NJ, nocoll,
                       w8, wema_sb, make_prefetch(_rep),
                       (lambda: None), bank)
    nc.finalize()
    return nc


def _emit_p1(nc, tc, mybir, AF, bf16, f32, fp8, mm, rt, wa, wd, w_in, wema,
             bsb, bias_ap, rema, rema8, w8, wema_sb, p_rt, p1w, handoff,
             bank):
    """Phase 1 (R_EMA). Emitted for rep i+1 between rep i's phase 4 and
    phase 5, so its 256 bf16 matmuls execute inside rep i's K/V-AllGather
    window — the attention phases never read rema/rema8, so no double
    buffering is needed and the gather latency is fully hidden."""
    if True:
        # ---------------- Phase 1: R_EMA (bf16) ----------------
        rt_sb = handoff.pop("rt", None)
        if rt_sb is None:
            rt_sb = p_rt.tile([P, FC, S + 1], bf16, tag="rt", name="rt0")
            nc.sync.dma_start(out=rt_sb[:, 0:FC // 2, :],
                              in_=rt.ap()[:, 0:FC // 2, :])
            nc.sync.dma_start(out=rt_sb[:, FC // 2:, :],
                              in_=rt.ap()[:, FC // 2:, :])
        with tc.tile_pool(name="p1t", bufs=2) as p1t:
            for ot in range(FC):
                if ot == 0 and "wa0" in handoff:
                    wa_t = handoff.pop("wa0")
                    wd_t = handoff.pop("wd0")
                else:
                    wa_t = p1w.tile([P, 2 * FC, P], bf16, tag="wa")
                    nc.sync.dma_start(out=wa_t, in_=wa.ap()[ot])
                    wd_t = p1w.tile([P, 2 * FC, P], bf16, tag="wd")
                    nc.sync.dma_start(out=wd_t, in_=wd.ap()[ot])
                # stagger the phase-2/4 weight prefetches behind the
                # early wa/wd loads so phase 1 starts immediately but
                # later GEMMs never wait on weights
                if ot == 1:
                    for h in range(2):
                        nc.sync.dma_start(out=w8["wz"][h],
                                          in_=w_in["wz"].ap()[h])
                    for h in range(2):
                        nc.sync.dma_start(out=w8["wk"][h],
                                          in_=w_in["wk"].ap()[h])
                elif ot == 3:
                    for h in range(2):
                        nc.sync.dma_start(out=w8["wv"][h],
                                          in_=w_in["wv"].ap()[h])
                    for h in range(2):
                        nc.sync.dma_start(out=w8["wq"][h],
                                          in_=w_in["wq"].ap()[h])
                elif ot == 5:
                    for h in range(2):
                        nc.sync.dma_start(out=w8["wf"][h],
                                          in_=w_in["wf"].ap()[h])
                elif ot == 6:
                    for h in range(2):
                        nc.sync.dma_start(out=wema_sb[h],
                                          in_=wema.ap()[h])
                ps_a = bank[1 + 2 * (ot % 3)]
                ps_d = bank[2 + 2 * (ot % 3)]
                for ch in range(FC):
                    mm(ps_a, wa_t[:, ch, :], rt_sb[:, ch, 0:S],
                       start=(ch == 0), stop=False)
                    mm(ps_d, wd_t[:, ch, :], rt_sb[:, ch, 0:S],
                       start=(ch == 0), stop=False)
                for ch in range(FC):
                    mm(ps_a, wa_t[:, FC + ch, :], rt_sb[:, ch, 1:S + 1],
                       start=False, stop=(ch == FC - 1))
                    mm(ps_d, wd_t[:, FC + ch, :], rt_sb[:, ch, 1:S + 1],
                       start=False, stop=(ch == FC - 1))
                alpha_t = p1t.tile([P, S], f32, tag="alpha", bufs=1)
                nc.scalar.activation(alpha_t, ps_a, AF.Tanh,
                                     bias=bias_ap(0, ot), scale=1.0)
                delta_t = p1t.tile([P, S], f32, tag="delta", bufs=1)
                nc.scalar.activation(delta_t, ps_d, AF.Tanh,
                                     bias=bias_ap(1, ot), scale=1.0)
                # rema = t1 + alpha*(r_t - t1), t1 = delta*r_prev
                t1 = p1t.tile([P, S], f32, tag="t1", bufs=1)
                nc.vector.tensor_mul(t1, delta_t, rt_sb[:, ot, 0:S])
                t2 = p1t.tile([P, S], f32, tag="t2", bufs=1)
                nc.vector.tensor_sub(t2, rt_sb[:, ot, 1:S + 1], t1)
                t3 = p1t.tile([P, S], f32, tag="t3", bufs=1)
                nc.vector.tensor_mul(t3, alpha_t, t2)
                nc.vector.tensor_add(rema[:, ot, :], t3, t1)
                nc.scalar.copy(rema8[:, ot, :], rema[:, ot, :])


def _emit_rest(nc, tc, mybir, AF, bf16, f32, fp8, mm, rt, wa, wd, w_in, wema,
               wi, wfin, out, row_bounce, kt_in, kt_out, v_in, v_out, bsb,
               bias_ap, ones_dr, lnsp_col, bv_b, rema, rema8, z8, qT8, remap,
               remap8, fT, zatp8, i_row, fin_rem, KT_ELEMS, V_ELEMS, NJ,
               nocoll, w8, wema_sb, prefetch_cb, emit_next_p1, bank):
    DR = mybir.MatmulPerfMode.DoubleRow
    Alu = mybir.AluOpType

    if True:
        # ---------------- Phase 2: Z, K.T, V + AllGather; then Q --------
        with tc.tile_pool(name="p_kv", bufs=1) as p_kv:
            ps_rot = [0]

            def next_ps():
                b = bank[1 + ps_rot[0] % 6]
                ps_rot[0] += 1
                return b
            def proj8(wname, rhs_src, out_tile, func, bias_row, scale,
                      half_done=None):
                for half in range(2):
                    w_t = w8[wname][half]
                    for sub in range(4):
                        ot = half * 4 + sub
                        ow = slice(sub * P, (sub + 1) * P)
                        ps = next_ps()
                        for cp in range(FC // 2):
                            mm(ps, w_t[:, 2 * cp:2 * cp + 2, ow],
                               rhs_src[:, 2 * cp:2 * cp + 2, :],
                               start=(cp == 0), stop=(cp == FC // 2 - 1),
                               perf_mode=DR)
                        if func is AF.Identity:
                            nc.vector.tensor_scalar(
                                out_tile[:, ot, :], ps, scale,
                                bias_ap(bias_row, ot), Alu.mult, Alu.add)
                        else:
                            nc.scalar.activation(out_tile[:, ot, :], ps,
                                                 func,
                                                 bias=bias_ap(bias_row, ot),
                                                 scale=scale)
                    if half_done is not None:
                        half_done(half)

            # z = silu(zpre) computed as u*(1+tanh(u)), u = zpre/2 — keeps
            # every activation in the tanh/exp table set (no ACT_TABLE_LOAD
            # between phases). bias row 2 holds b_z/2 (host-packed).
            for half in range(2):
                w_t = w8["wz"][half]
                for sub in range(4):
                    ot = half * 4 + sub
                    ow = slice(sub * P, (sub + 1) * P)
                    ps = next_ps()
                    for cp in range(FC // 2):
                        mm(ps, w_t[:, 2 * cp:2 * cp + 2, ow],
                           rema8[:, 2 * cp:2 * cp + 2, :],
                           start=(cp == 0), stop=(cp == FC // 2 - 1),
                           perf_mode=DR)
                    u_t = p_kv.tile([P, S], f32, tag="uz", bufs=2)
                    nc.vector.tensor_scalar(u_t, ps, 0.5 / S_W,
                                            bias_ap(2, ot),
                                            Alu.mult, Alu.add)
                    t_t = p_kv.tile([P, S], f32, tag="tz", bufs=2)
                    nc.scalar.activation(t_t, ps, AF.Tanh,
                                         bias=bias_ap(2, ot),
                                         scale=0.5 / S_W)
                    nc.vector.scalar_tensor_tensor(
                        z8[:, ot, :], t_t, 1.0, u_t, Alu.add, Alu.mult)

            # K.T -> kt_in (feature-major, partition-contiguous), gather
            # ASAP; half-0 is stored while half-1 computes so the gather
            # kick only waits on the second 0.25MB store
            ktS = p_kv.tile([P, FC, S], fp8)
            kt_dram = kt_in[:].rearrange("(p c s) -> p c s", p=P, s=S)
            proj8("wk", z8, ktS, AF.Identity, 4, 1.0 / S_W,
                  half_done=lambda h: nc.sync.dma_start(
                      out=kt_dram[:, h * 4:(h + 1) * 4, :],
                      in_=ktS[:, h * 4:(h + 1) * 4, :]))
            if not nocoll:
                nc.gpsimd.collective_compute(
                    "AllGather", mybir.AluOpType.bypass,
                    replica_groups=[list(range(NCORES))],
                    ins=[kt_in[:].opt()], outs=[kt_out[:].opt()],
                )
            # next rep's rt / first wa+wd loads: issued after the gather
            # kick so they never delay it, ~50us before phase 1 needs them
            prefetch_cb()

            # V seq-major: V[s, o] = sum_d Z.T[d, s] Wv.T[d, o]; x S_V + bias
            for half in range(2):
                osl = slice(half * 4 * P, (half + 1) * 4 * P)
                wv_t = w8["wv"][half]
                for st in range(4):
                    ssl = slice(st * P, (st + 1) * P)
                    ps = next_ps()[:, 0:4 * P]
                    for cp in range(FC // 2):
                        mm(ps, z8[:, 2 * cp:2 * cp + 2, ssl],
                           wv_t[:, 2 * cp:2 * cp + 2, :],
                           start=(cp == 0), stop=(cp == FC // 2 - 1),
                           perf_mode=DR)
                    v_sb = p_kv.tile([P, 4 * P], fp8, tag="vsb")
                    # v8 = ps*(S_V/S_W) + S_V*b_v  (bv_b pre-scaled on host)
                    nc.vector.scalar_tensor_tensor(
                        v_sb, ps, S_V / S_W, bv_b[:, osl],
                        Alu.mult, Alu.add)
                    nc.sync.dma_start(
                        out=v_in[st * P * D:(st + 1) * P * D].rearrange(
                            "(p o) -> p o", p=P)[:, osl],
                        in_=v_sb)
            if not nocoll:
                nc.gpsimd.collective_compute(
                    "AllGather", mybir.AluOpType.bypass,
                    replica_groups=[list(range(NCORES))],
                    ins=[v_in[:].opt()], outs=[v_out[:].opt()],
                )

            # Q (overlaps the AllGathers)
            proj8("wq", z8, qT8, AF.Identity, 3, 1.0 / S_W)

            # -------- Phase 4: R_EMA' (bf16), f, i (overlap AllGather) ---
            def proj_bf(w_halves, rhs_src, out_tile, func, bias_row):
                for half in range(2):
                    w_t = w_halves[half]
                    for sub in range(4):
                        ot = half * 4 + sub
                        ow = slice(sub * P, (sub + 1) * P)
                        ps = next_ps()
                        for ch in range(FC):
                            mm(ps, w_t[:, ch, ow], rhs_src[:, ch, :],
                               start=(ch == 0), stop=(ch == FC - 1))
                        nc.scalar.activation(out_tile[:, ot, :], ps, func,
                                             bias=bias_ap(bias_row, ot),
                                             scale=1.0)
                        if out_tile is remap:
                            nc.scalar.copy(remap8[:, ot, :],
                                           remap[:, ot, :])

            proj_bf(wema_sb, rema, remap, AF.Identity, 6)
            # f-gate: fT holds tanh(fpre/2); f = 0.5*(1+fT) is folded into
            # the 1/l factor later (ones_dr=2 pre-doubles the denominator).
            # bias row 7 holds b_f/2 (host-packed).
            proj8("wf", remap8, fT, AF.Tanh, 7, 0.5 / S_W)

            wi_sb = p_kv.tile([P, FC], bf16, tag="wi")
            nc.sync.dma_start(out=wi_sb, in_=wi.ap())
            wfin_sb = p_kv.tile([P, FC], bf16, tag="wfin")
            nc.sync.dma_start(out=wfin_sb, in_=wfin.ap())
            ps_i = bank[7][0:1, :]
            for ch in range(FC):
                mm(ps_i, wi_sb[:, ch:ch + 1], rema[:, ch, :],
                   start=(ch == 0), stop=(ch == FC - 1))
            nc.scalar.activation(i_row, ps_i, AF.Tanh,
                                 bias=bsb[0:1, 9, 0:1], scale=1.0)

            # fin_rem = remap @ W_final.T  (the (1-i) branch of the head)
            ps_fr = bank[0][0:1, :]
            for ch in range(FC):
                mm(ps_fr, wfin_sb[:, ch:ch + 1], remap[:, ch, :],
                   start=(ch == 0), stop=(ch == FC - 1))
            nc.scalar.copy(fin_rem, ps_fr)

    # next rep's phase 1: its matmuls fill the PE while this rep's
    # K/V AllGather completes
    emit_next_p1()

    # ---------------- Phase 5+6: attention + head ----------------
    # One PSUM pool of 8 named bank tiles shared by 5A (score chains rotate
    # banks 0-2, denominator lives in a bank-3 row), 5B (zat accumulators in
    # all 8) and the head (chains reuse each bank as its zatp drain
    # completes; the final row-matmul reuses a bank-0 row). Bank-granular
    # tile reuse gives WAR deps instead of pool-alloc barriers, so each
    # stage starts as soon as its first operand is ready.
    with tc.tile_pool(name="p_pt", bufs=1) as p_pt, \
         tc.tile_pool(name="p6t", bufs=1) as p6t, \
         tc.tile_pool(name="p6w", bufs=1) as p6w:
        pt = p_pt.tile([P, NJ, S], fp8)
        l_ps = bank[3][0:1, :]
        wzat_t = [p6w.tile([P, FC, 4 * P], fp8, name=f"wzat{h}")
                  for h in range(2)]
        wfin_sb6 = p6t.tile([P, FC], bf16, tag="wfin6")
        nc.sync.dma_start(out=wfin_sb6, in_=wfin.ap())
        junk = p6t.tile([P, 1], f32, tag="junk")
        # preload the exp table set while waiting on the K.T AllGather
        nc.scalar.activation(junk, bsb[:, 0, 0:1], AF.Exp, bias=0.0,
                             scale=0.0)

        # --- 5A: scores.T + exp; denominator trails one rank behind ---
        def l_pairs(r, start, stop):
            for h in range(2):
                mm(l_ps, ones_dr[:, :, 0:1],
                   pt[:, 4 * r + 2 * h:4 * r + 2 * h + 2, :],
                   start=(start and h == 0), stop=(stop and h == 1),
                   perf_mode=DR)

        with tc.tile_pool(name="p5w", bufs=2) as p5w:
            for r in range(NCORES):
                kt_r = p5w.tile([P, FC, S], fp8, tag="ktr")
                kt_src = kt_in[:] if nocoll else kt_out[r]
                kt_ap = kt_src.rearrange("(p c s) -> p c s", p=P, s=S)
                nc.sync.dma_start(out=kt_r[:, 0:FC // 2, :],
                                  in_=kt_ap[:, 0:FC // 2, :])
                nc.sync.dma_start(out=kt_r[:, FC // 2:, :],
                                  in_=kt_ap[:, FC // 2:, :])
                if r == 4:
                    for h in range(2):
                        nc.sync.dma_start(out=wzat_t[h],
                                          in_=w_in["wzat"].ap()[h])
                for cl in range(4):
                    jc = r * 4 + cl
                    jw = slice(cl * P, (cl + 1) * P)
                    s_ps = bank[(r * 4 + cl) % 3]
                    for cp in range(FC // 2):
                        mm(s_ps, kt_r[:, 2 * cp:2 * cp + 2, jw],
                           qT8[:, 2 * cp:2 * cp + 2, :],
                           start=(cp == 0), stop=(cp == FC // 2 - 1),
                           perf_mode=DR)
                    # pt = S_P * exp(score): ln(S_P) folded into the bias
                    nc.scalar.activation(pt[:, jc, :], s_ps, AF.Exp,
                                         bias=lnsp_col, scale=ATT_SCALE)
                # denominator pairs for the previous rank: their exps are
                # long done, so these never stall the in-order PE queue
                if r >= 1:
                    l_pairs(r - 1, start=(r == 1), stop=False)
            l_pairs(NCORES - 1, start=False, stop=True)

        # preload the tanh set for the head; all exps are already queued
        nc.scalar.activation(junk, bsb[:, 0, 0:1], AF.Tanh, bias=0.0,
                             scale=0.0)

        with tc.tile_pool(name="p5v", bufs=2) as p5v:
            # prefetch rank-0 V before the reciprocal chain so its DMAs
            # are not queued behind the bounce round-trip
            v_r0 = p5v.tile([P, 4, D], fp8, tag="vr")
            v_src0 = v_in[:] if nocoll else v_out[0]
            v_ap0 = v_src0.rearrange("(t p o) -> p t o", p=P, o=D)
            nc.sync.dma_start(out=v_r0[:, 0:2, :], in_=v_ap0[:, 0:2, :])
            nc.sync.dma_start(out=v_r0[:, 2:4, :], in_=v_ap0[:, 2:4, :])


            # l_ps = 2*S_P*l (ones_dr=2), so rl = 0.5/(S_P*l); the 0.5 is
            # the sigmoid-via-tanh constant of the f gate. ~18-bit
            # reciprocal is plenty for a uniform per-column softmax scale.
            l_row = p6t.tile([1, S], f32, tag="lrow")
            nc.vector.reciprocal_approx_fast(l_row, l_ps)
            nc.sync.dma_start(out=row_bounce[0:1, :], in_=l_row)
            rl_b = p6t.tile([P, S], f32, tag="rlb")
            nc.sync.dma_start(out=rl_b,
                              in_=row_bounce[0:1, :].partition_broadcast(P))
            # fT <- (tanh(fpre/2) + 1) * 0.5/(S_P*l)  ==  f/(S_P*l)
            for ot in range(FC):
                nc.vector.scalar_tensor_tensor(
                    fT[:, ot, :], fT[:, ot, :], 1.0, rl_b,
                    Alu.add, Alu.mult)

            # --- 5B: Zat.T (whole-rank V loads) ---
            for r in range(NCORES):
                if r == 0:
                    v_r = v_r0
                else:
                    v_r = p5v.tile([P, 4, D], fp8, tag="vr")
                    v_src = v_in[:] if nocoll else v_out[r]
                    v_ap = v_src.rearrange("(t p o) -> p t o", p=P, o=D)
                    nc.sync.dma_start(out=v_r[:, 0:2, :],
                                      in_=v_ap[:, 0:2, :])
                    nc.sync.dma_start(out=v_r[:, 2:4, :],
                                      in_=v_ap[:, 2:4, :])
                for u in range(2):
                    jp = r * 2 + u
                    # first group: bank 3 last, so its WAR on the
                    # denominator-reciprocal read never stalls the queue
                    ots = [0, 1, 2, 4, 5, 6, 7, 3] if jp == 0 else range(FC)
                    for ot in ots:
                        mm(bank[ot],
                           v_r[:, 2 * u:2 * u + 2, ot * P:(ot + 1) * P],
                           pt[:, 4 * r + 2 * u:4 * r + 2 * u + 2, :],
                           start=(jp == 0), stop=(jp == NJ // 2 - 1),
                           perf_mode=DR)

        # ---------------- Phase 6: output head ----------------
        # p_hat = sigmoid(i*(Ztp@Wf) + (1-i)*(remap@Wf)); remap@Wf in ph 4.
        # zatp drain, head chain and tanh pipeline per-bank: chain ot
        # starts as soon as bank ot is drained, while later banks finish.
        ztps = []
        for ot in range(FC):
            nc.vector.tensor_mul(zatp8[:, ot, :], bank[ot], fT[:, ot, :])
        for half in range(2):
            w_t = wzat_t[half]
            for sub in range(4):
                ot = half * 4 + sub
                ow = slice(sub * P, (sub + 1) * P)
                ps = bank[ot]
                for cp in range(FC // 2):
                    mm(ps, w_t[:, 2 * cp:2 * cp + 2, ow],
                       zatp8[:, 2 * cp:2 * cp + 2, :],
                       start=(cp == 0), stop=(cp == FC // 2 - 1),
                       perf_mode=DR)
                t_sum = p6t.tile([P, S], f32, tag="tsum", bufs=2)
                # t_sum = ps/(S_V*S_W) + remap
                nc.vector.scalar_tensor_tensor(
                    t_sum, ps, 1.0 / (S_V * S_W), remap[:, ot, :],
                    Alu.mult, Alu.add)
                ztp = p6t.tile([P, S], bf16, tag=f"ztp{ot}",
                               name=f"ztp{ot}")
                nc.scalar.activation(ztp, t_sum, AF.Tanh,
                                     bias=bias_ap(8, ot), scale=1.0)
                ztps.append(ztp)
        # fin matmuls after all tanh tiles: no per-tile PE stall on ACT.
        # The [1,S] accumulator reuses a bank-0 row (bank 0 is long drained).
        fin_ps = bank[0][0:1, :]
        for ot in range(FC):
            mm(fin_ps, wfin_sb6[:, ot:ot + 1], ztps[ot],
               start=(ot == 0), stop=(ot == FC - 1))
        # blend the two head branches on the [1, S] rows
        d_r = p6t.tile([1, S], f32, tag="dr")
        nc.vector.tensor_sub(d_r, fin_ps, fin_rem)
        m_r = p6t.tile([1, S], f32, tag="mr")
        nc.vector.tensor_mul(m_r, d_r, i_row)
        s_r = p6t.tile([1, S], f32, tag="sr")
        nc.vector.tensor_add(s_r, m_r, fin_rem)
        # sigmoid via tanh: p = 0.5*(1 + tanh(x/2)) — stays in the tanh set
        t_fin = p6t.tile([1, S], f32, tag="tfin")
        nc.scalar.activation(t_fin, s_r, AF.Tanh, bias=0.0, scale=0.5)
        phat = p6t.tile([1, S], f32, tag="phat")
        nc.vector.tensor_scalar(phat, t_fin, 0.5, 0.5, Alu.mult, Alu.add)
        nc.sync.dma_start(out=out.ap().rearrange("s o -> o s"), in_=phat)


def _prep_host_inputs(inputs):
    """Transpose/pack weights into SBUF layouts + build per-core shards."""
    import concourse.mybir as mybir
    bf16 = mybir.dt.np(mybir.dt.bfloat16)
    fp8 = mybir.dt.np(mybir.dt.float8e4)

    R = np.ascontiguousarray(inputs["R"], dtype=np.float32)
    RT_ext = np.concatenate(
        [np.zeros((D, 1), np.float32), np.ascontiguousarray(R.T)], axis=1)

    def pack_p1(w):  # W [D, 2D] -> W.T [2D, D] -> [ot, p, k(2FC), o(P)]
        wt = np.ascontiguousarray(np.asarray(w, np.float32).T)
        return np.ascontiguousarray(
            wt.reshape(2 * FC, P, FC, P).transpose(2, 1, 0, 3)).astype(bf16)

    def pack_proj(w, dt, scale=1.0):
        # W [D, D] -> W.T [D, D] -> [half, p, c(FC), o(4P)]
        wt = np.ascontiguousarray(np.asarray(w, np.float32).T * scale)
        return np.ascontiguousarray(
            wt.reshape(FC, P, 2, 4 * P).transpose(2, 1, 0, 3)).astype(dt)

    def pack_col(w):  # W [1, D] -> [p, c(FC)]
        wt = np.asarray(w, np.float32).reshape(FC, P).T
        return np.ascontiguousarray(wt).astype(bf16)

    w = {
        "wa": pack_p1(inputs["W_alpha"]), "wd": pack_p1(inputs["W_delta"]),
        "wz": pack_proj(inputs["W_z"], fp8, S_W),
        "wq": pack_proj(inputs["W_q"], fp8, S_W),
        "wk": pack_proj(inputs["W_k"], fp8, S_W),
        "wv": pack_proj(inputs["W_v"], fp8, S_W),
        "wf": pack_proj(inputs["W_f"], fp8, S_W),
        "wzat": pack_proj(inputs["W_z_at"], fp8, S_W),
        "wema": pack_proj(inputs["W_EMA"], bf16),
        "wi": pack_col(inputs["W_i"]), "wfin": pack_col(inputs["W_final"]),
    }

    brows = np.zeros((10, D), np.float32)
    brows[0] = inputs["b_alpha"]
    brows[1] = inputs["b_delta"]
    brows[2] = np.asarray(inputs["b_z"]) * 0.5   # silu-via-tanh uses b_z/2
    brows[3] = inputs["b_q"]
    brows[4] = inputs["b_k"]
    brows[5] = inputs["b_v"]
    brows[6] = inputs["b_EMA"]
    brows[7] = np.asarray(inputs["b_f"]) * 0.5   # sigmoid-via-tanh: b_f/2
    brows[8] = inputs["b_z_at"]
    brows[9, 0] = np.float32(inputs["b_i"][0])
    biases = np.ascontiguousarray(
        brows.reshape(10, FC, P).transpose(2, 0, 1))  # [p, 10, FC]
    bvrow = np.ascontiguousarray(brows[5:6] * S_V)  # [1, D], pre-scaled

    in_maps = []
    for c in range(NCORES):
        rt_c = RT_ext[:, c * S:c * S + S + 1]  # [D, S+1]
        rt_p = np.ascontiguousarray(
            rt_c.reshape(FC, P, S + 1).transpose(1, 0, 2)).astype(bf16)
        m = {"rt": rt_p, "biases": biases, "bvrow": bvrow}
        m.update(w)
        in_maps.append(m)
    return in_maps


def kernel(**inputs):
    from concourse.bass_utils import run_bass_kernel_spmd

    if "nc" not in _CACHE:
        _CACHE["nc"] = _build_bass()
    nc = _CACHE["nc"]
    in_maps = _prep_host_inputs(inputs)
    res = run_bass_kernel_spmd(nc, in_maps, core_ids=list(range(NCORES)))
    outs = [res.results[c]["out"] for c in range(NCORES)]
    return np.concatenate(outs, axis=0).astype(np.float32)


# revision 44
# speedup vs baseline: 1.1077x; 1.1077x over previous
"""MEGADecoder forward pass as a Bass/Tile kernel on 8 TRN2 NeuronCores.

Sharding: sequence-parallel. Each core owns SEQ/8 = 512 rows. Params are
replicated. Single-head full attention uses one AllGather of (K.T, V).

Layout: activations are stored feature-major ([8 chunks x 128 partitions,
seq 512 free]) so every GEMM is a chain of PE matmuls with no transposes:
  - projections:  out.T[o, s] = sum_d W.T[d, o] . act.T[d, s]
  - V projection: V[s, o]     = sum_d Z.T[d, s] . Wv.T[d, o]   (seq-major out)
  - scores.T:     S.T[j, i]   = sum_f K.T[f, j] . Q.T[f, i]
  - attention:    Zat.T[o, s] = sum_j V[j, o]   . P.T[j, s]
Softmax runs without max-subtraction (scores are O(1)); the denominator
accumulates via DoubleRow ones-matmuls that trail the score chains by one
rank (so they never stall the in-order PE queue), its ~18-bit fast
reciprocal bounces through DRAM for the partition broadcast, and 1/l (with
the sigmoid-via-tanh 0.5 pre-folded via ones=2) lands in the attention
gate f off the critical path during 5B.

Scheduling: the 8 physical PSUM banks are global named tiles shared by
every phase and every benchmark rep — subtile WAR deps replace pool-alloc
barriers, so each stage (and the next rep's phase 1) starts the moment the
individual bank it needs drains. Weight tiles persist across reps with
reload DMAs staggered through phase 1; the next rep's rt/wa/wd loads are
emitted mid-phase-5. Every activation stays in one ACT table set (silu and
all sigmoids are computed as tanh identities with constants folded into
existing drains; the two exp/tanh set loads are preloaded by dummy
activations in hidden windows). q/k Identity drains run on DVE to unload
the scalar engine; K.T stores to DRAM in halves so its AllGather kicks
~2us earlier.

fp8 (e4m3) + DoubleRow: every GEMM except phase-1 (alpha/delta), W_EMA and
the tiny heads runs with both operands in fp8e4 and perf_mode=DoubleRow
(256-deep contraction per instruction, ~1.4x the bf16 matmul rate).
Numerics were validated against a numpy bit-model of this exact pipeline:
bf16 everywhere = 7.86e-3 max-rel error, this fp8 split = 8.2e-3 (tolerance
2e-2); putting phase-1 or W_EMA in fp8 blows up to 6e-2, so those stay bf16.
Scales: weights x32 (their 0.054 max would otherwise sit in subnormals),
V x16, pt = 8*exp(s) via an ln(8) bias folded into the Exp activation;
activations are stored unscaled (std ~0.5-1.2 is mid-range for e4m3).
All descales fold into existing activation/DVE drains. TRN fp8 e4m3
overflows to Inf above 240 (no saturation): measured maxes are pt 120,
V*16 64, zatp*16 12, weights*32 1.73.

The K.T / V AllGather payloads are fp8 (0.5 MB in, 4 MB out per tensor),
and phase 5 re-reads 8 MB instead of 16 MB. Weight loads are issued early
inside the phase-1 loop (wz/wk at ot==1, wv/wq at ot==3, wf at ot==5,
wema at ot==6) so no GEMM waits on its weights.
"""

import numpy as np

SEQ = 4096
D = 1024
NCORES = 8
S = SEQ // NCORES  # 512 rows per core
P = 128
FC = D // P  # 8 feature chunks
ATT_SCALE = 1.0 / float(np.sqrt(np.float32(D)))
S_W = 32.0   # fp8 weight scale
S_V = 16.0   # fp8 V scale
S_P = 8.0    # pt = S_P * exp(score)
LN_SP = float(np.log(S_P))

_CACHE = {}


def _build_bass(reps=1, nocoll=False):
    import concourse.bacc as bacc
    import concourse.tile as tile
    import concourse.mybir as mybir

    f32 = mybir.dt.float32
    bf16 = mybir.dt.bfloat16
    fp8 = mybir.dt.float8e4
    AF = mybir.ActivationFunctionType

    nc = bacc.Bacc(None, target_bir_lowering=False, num_devices=NCORES)
    mm = nc.tensor.matmul

    # ---- DRAM I/O (all host-packed layouts) ----
    rt = nc.dram_tensor("rt", [P, FC, S + 1], bf16, kind="ExternalInput")
    # phase-1 weights: [ot, p, k(2FC), o(P)]
    wa = nc.dram_tensor("wa", [FC, P, 2 * FC, P], bf16, kind="ExternalInput")
    wd = nc.dram_tensor("wd", [FC, P, 2 * FC, P], bf16, kind="ExternalInput")
    # fp8 projection weights (x32): [half, p, c(FC), o(4P)]
    w_in = {}
    for name in ["wz", "wq", "wk", "wv", "wf", "wzat"]:
        w_in[name] = nc.dram_tensor(name, [2, P, FC, 4 * P], fp8,
                                    kind="ExternalInput")
    wema = nc.dram_tensor("wema", [2, P, FC, 4 * P], bf16,
                          kind="ExternalInput")
    wi = nc.dram_tensor("wi", [P, FC], bf16, kind="ExternalInput")
    wfin = nc.dram_tensor("wfin", [P, FC], bf16, kind="ExternalInput")
    # biases packed [p, 10, FC]: rows alpha,delta,z,q,k,v,ema,f,zat,i
    biases = nc.dram_tensor("biases", [P, 10, FC], f32, kind="ExternalInput")
    bvrow = nc.dram_tensor("bvrow", [1, D], f32, kind="ExternalInput")
    out = nc.dram_tensor("out", [S, 1], f32, kind="ExternalOutput")

    KT_ELEMS = D * S
    V_ELEMS = S * D
    NJ = SEQ // P  # 32 j-chunks

    with tile.TileContext(nc) as tc, \
         tc.tile_pool(name="consts", bufs=1) as consts, \
         tc.tile_pool(name="dram", bufs=1, space="DRAM") as dram, \
         tc.tile_pool(name="big", bufs=1) as big, \
         tc.tile_pool(name="pw8", bufs=1) as pw8, \
         tc.tile_pool(name="p_rt", bufs=2) as p_rt, \
         tc.tile_pool(name="p1w", bufs=2) as p1w, \
         tc.tile_pool(name="gps", bufs=1, space="PSUM") as gps:

        bsb = consts.tile([P, 10, FC], f32)
        nc.sync.dma_start(out=bsb, in_=biases.ap())
        ones_dr = consts.tile([P, 2, 16], fp8)
        nc.vector.memset(ones_dr, 2.0)  # folds the 0.5 of sigmoid-via-tanh
        lnsp_col = consts.tile([P, 1], f32)
        nc.vector.memset(lnsp_col, LN_SP)
        bv_b = consts.tile([P, D], f32)
        nc.sync.dma_start(out=bv_b, in_=bvrow.ap().partition_broadcast(P))

        def bias_ap(row, chunk):
            return bsb[:, row, chunk:chunk + 1]

        # persistent weight tiles: hoisted out of the rep loop so rep i+1's
        # reloads only WAR rep i's last reader of the same tile (mid-rep),
        # instead of hitting a pool-alloc barrier at rep end.
        w8 = {}
        for name in ["wz", "wk", "wv", "wq", "wf", "wzat"]:
            w8[name] = [pw8.tile([P, FC, 4 * P], fp8, name=f"{name}{h}")
                        for h in range(2)]
        wema_sb = [pw8.tile([P, FC, 4 * P], bf16, name=f"wema{h}")
                   for h in range(2)]

        # the 8 physical PSUM banks as global tiles, shared by every
        # phase and rep: subtile WAR/RAW deps replace pool-alloc barriers,
        # so e.g. rep i+1's phase-1 chains start as soon as rep i's head
        # drains the individual bank, not when the whole head finishes
        bank = [gps.tile([P, S], f32, name=f"bank{i}") for i in range(FC)]

        # whole-kernel resident activations
        rema = big.tile([P, FC, S], bf16, name="rema")
        rema8 = big.tile([P, FC, S], fp8, name="rema8")
        z8 = big.tile([P, FC, S], fp8, name="z8")
        qT8 = big.tile([P, FC, S], fp8, name="qT8")
        remap = big.tile([P, FC, S], bf16, name="remap")
        remap8 = big.tile([P, FC, S], fp8, name="remap8")
        fT = big.tile([P, FC, S], bf16, name="fT")
        zatp8 = big.tile([P, FC, S], fp8, name="zatp8")
        i_row = big.tile([1, S], f32, name="i_row")
        fin_rem = big.tile([1, S], f32, name="fin_rem")

        # next-rep input prefetch: rep i+1's rt / first wa+wd loads are
        # emitted mid-way through rep i's phase 5, so the next iteration's
        # phase 1 starts with its inputs already resident instead of
        # queueing its DMAs behind rep i's entire stream.
        handoff = {}

        def make_prefetch(rep_idx):
            def cb():
                if rep_idx + 1 >= reps:
                    return
                nrt = p_rt.tile([P, FC, S + 1], bf16, tag="rt",
                                name=f"rt{rep_idx + 1}")
                nc.sync.dma_start(out=nrt[:, 0:FC // 2, :],
                                  in_=rt.ap()[:, 0:FC // 2, :])
                nc.sync.dma_start(out=nrt[:, FC // 2:, :],
                                  in_=rt.ap()[:, FC // 2:, :])
                nwa = p1w.tile([P, 2 * FC, P], bf16, tag="wa",
                               name=f"wa0_{rep_idx + 1}")
                nc.sync.dma_start(out=nwa, in_=wa.ap()[0])
                nwd = p1w.tile([P, 2 * FC, P], bf16, tag="wd",
                               name=f"wd0_{rep_idx + 1}")
                nc.sync.dma_start(out=nwd, in_=wd.ap()[0])
                handoff["rt"] = nrt
                handoff["wa0"] = nwa
                handoff["wd0"] = nwd
            return cb

        def emit_p1_for(rep_idx):
            def cb():
                if rep_idx >= reps:
                    return
                _emit_p1(nc, tc, mybir, AF, bf16, f32, fp8, mm, rt, wa, wd,
                         w_in, wema, bsb, bias_ap, rema, rema8, w8, wema_sb,
                         p_rt, p1w, handoff, bank)
            return cb

        for _rep in range(reps):
            emit_p1_for(_rep)()
            row_bounce = dram.tile([1, S], f32, name=f"rb{_rep}")
            kt_in = dram.tile([KT_ELEMS], fp8, name=f"kti{_rep}")
            v_in = dram.tile([V_ELEMS], fp8, name=f"vi{_rep}")
            if nocoll:
                kt_out = v_out = None
            else:
                kt_out = dram.tile([NCORES, KT_ELEMS], fp8,
                                   addr_space="Shared", name=f"kto{_rep}")
                v_out = dram.tile([NCORES, V_ELEMS], fp8,
                                  addr_space="Shared", name=f"vo{_rep}")
            _emit_rest(nc, tc, mybir, AF, bf16, f32, fp8, mm, rt, wa, wd,
                       w_in, wema, wi, wfin, out, row_bounce, kt_in, kt_out,
                       v_in, v_out, bsb, bias_ap, ones_dr, lnsp_col, bv_b,
                       rema, rema8, z8, qT8, remap, remap8, fT, zatp8, i_row,
                       fin_rem, KT_ELEMS, V_ELEMS, NJ, nocoll,
                       w8, wema_sb, make_prefetch(_rep),
                       (lambda: None), bank)
    nc.finalize()
    return nc


def _emit_p1(nc, tc, mybir, AF, bf16, f32, fp8, mm, rt, wa, wd, w_in, wema,
             bsb, bias_ap, rema, rema8, w8, wema_sb, p_rt, p1w, handoff,
             bank):
    """Phase 1 (R_EMA). Emitted for rep i+1 between rep i's phase 4 and
    phase 5, so its 256 bf16 matmuls execute inside rep i's K/V-AllGather
    window — the attention phases never read rema/rema8, so no double
    buffering is needed and the gather latency is fully hidden."""
    if True:
        # ---------------- Phase 1: R_EMA (bf16) ----------------
        rt_sb = handoff.pop("rt", None)
        if rt_sb is None:
            rt_sb = p_rt.tile([P, FC, S + 1], bf16, tag="rt", name="rt0")
            nc.sync.dma_start(out=rt_sb[:, 0:FC // 2, :],
                              in_=rt.ap()[:, 0:FC // 2, :])
            nc.sync.dma_start(out=rt_sb[:, FC // 2:, :],
                              in_=rt.ap()[:, FC // 2:, :])
        with tc.tile_pool(name="p1t", bufs=2) as p1t:
            for ot in range(FC):
                if ot == 0 and "wa0" in handoff:
                    wa_t = handoff.pop("wa0")
                    wd_t = handoff.pop("wd0")
                else:
                    wa_t = p1w.tile([P, 2 * FC, P], bf16, tag="wa")
                    nc.sync.dma_start(out=wa_t, in_=wa.ap()[ot])
                    wd_t = p1w.tile([P, 2 * FC, P], bf16, tag="wd")
                    nc.sync.dma_start(out=wd_t, in_=wd.ap()[ot])
                # stagger the phase-2/4 weight prefetches behind the
                # early wa/wd loads so phase 1 starts immediately but
                # later GEMMs never wait on weights
                if ot == 1:
                    for h in range(2):
                        nc.sync.dma_start(out=w8["wz"][h],
                                          in_=w_in["wz"].ap()[h])
                    for h in range(2):
                        nc.sync.dma_start(out=w8["wk"][h],
                                          in_=w_in["wk"].ap()[h])
                elif ot == 3:
                    for h in range(2):
                        nc.sync.dma_start(out=w8["wv"][h],
                                          in_=w_in["wv"].ap()[h])
                    for h in range(2):
                        nc.sync.dma_start(out=w8["wq"][h],
                                          in_=w_in["wq"].ap()[h])
                elif ot == 5:
                    for h in range(2):
                        nc.sync.dma_start(out=w8["wf"][h],
                                          in_=w_in["wf"].ap()[h])
                elif ot == 6:
                    for h in range(2):
                        nc.sync.dma_start(out=wema_sb[h],
                                          in_=wema.ap()[h])
                ps_a = bank[1 + 2 * (ot % 3)]
                ps_d = bank[2 + 2 * (ot % 3)]
                for ch in range(FC):
                    mm(ps_a, wa_t[:, ch, :], rt_sb[:, ch, 0:S],
                       start=(ch == 0), stop=False)
                    mm(ps_d, wd_t[:, ch, :], rt_sb[:, ch, 0:S],
                       start=(ch == 0), stop=False)
                for ch in range(FC):
                    mm(ps_a, wa_t[:, FC + ch, :], rt_sb[:, ch, 1:S + 1],
                       start=False, stop=(ch == FC - 1))
                    mm(ps_d, wd_t[:, FC + ch, :], rt_sb[:, ch, 1:S + 1],
                       start=False, stop=(ch == FC - 1))
                alpha_t = p1t.tile([P, S], f32, tag="alpha", bufs=1)
                nc.scalar.activation(alpha_t, ps_a, AF.Tanh,
                                     bias=bias_ap(0, ot), scale=1.0)
                delta_t = p1t.tile([P, S], f32, tag="delta", bufs=1)
                nc.scalar.activation(delta_t, ps_d, AF.Tanh,
                                     bias=bias_ap(1, ot), scale=1.0)
                # rema = t1 + alpha*(r_t - t1), t1 = delta*r_prev
                t1 = p1t.tile([P, S], f32, tag="t1", bufs=1)
                nc.vector.tensor_mul(t1, delta_t, rt_sb[:, ot, 0:S])
                t2 = p1t.tile([P, S], f32, tag="t2", bufs=1)
                nc.vector.tensor_sub(t2, rt_sb[:, ot, 1:S + 1], t1)
                t3 = p1t.tile([P, S], f32, tag="t3", bufs=1)
                nc.vector.tensor_mul(t3, alpha_t, t2)
                nc.vector.tensor_add(rema[:, ot, :], t3, t1)
                nc.scalar.copy(rema8[:, ot, :], rema[:, ot, :])


def _emit_rest(nc, tc, mybir, AF, bf16, f32, fp8, mm, rt, wa, wd, w_in, wema,
               wi, wfin, out, row_bounce, kt_in, kt_out, v_in, v_out, bsb,
               bias_ap, ones_dr, lnsp_col, bv_b, rema, rema8, z8, qT8, remap,
               remap8, fT, zatp8, i_row, fin_rem, KT_ELEMS, V_ELEMS, NJ,
               nocoll, w8, wema_sb, prefetch_cb, emit_next_p1, bank):
    DR = mybir.MatmulPerfMode.DoubleRow
    Alu = mybir.AluOpType

    if True:
        # ---------------- Phase 2: Z, K.T, V + AllGather; then Q --------
        with tc.tile_pool(name="p_kv", bufs=1) as p_kv:
            ps_rot = [0]

            def next_ps():
                b = bank[1 + ps_rot[0] % 6]
                ps_rot[0] += 1
                return b
            def proj8(wname, rhs_src, out_tile, func, bias_row, scale,
                      half_done=None):
                for half in range(2):
                    w_t = w8[wname][half]
                    for sub in range(4):
                        ot = half * 4 + sub
                        ow = slice(sub * P, (sub + 1) * P)
                        ps = next_ps()
                        for cp in range(FC // 2):
                            mm(ps, w_t[:, 2 * cp:2 * cp + 2, ow],
                               rhs_src[:, 2 * cp:2 * cp + 2, :],
                               start=(cp == 0), stop=(cp == FC // 2 - 1),
                               perf_mode=DR)
                        if func is AF.Identity:
                            nc.vector.tensor_scalar(
                                out_tile[:, ot, :], ps, scale,
                                bias_ap(bias_row, ot), Alu.mult, Alu.add)
                        else:
                            nc.scalar.activation(out_tile[:, ot, :], ps,
                                                 func,
                                                 bias=bias_ap(bias_row, ot),
                                                 scale=scale)
                    if half_done is not None:
                        half_done(half)

            # z = silu(zpre) computed as u*(1+tanh(u)), u = zpre/2 — keeps
            # every activation in the tanh/exp table set (no ACT_TABLE_LOAD
            # between phases). bias row 2 holds b_z/2 (host-packed).
            for half in range(2):
                w_t = w8["wz"][half]
                for sub in range(4):
                    ot = half * 4 + sub
                    ow = slice(sub * P, (sub + 1) * P)
                    ps = next_ps()
                    for cp in range(FC // 2):
                        mm(ps, w_t[:, 2 * cp:2 * cp + 2, ow],
                           rema8[:, 2 * cp:2 * cp + 2, :],
                           start=(cp == 0), stop=(cp == FC // 2 - 1),
                           perf_mode=DR)
                    u_t = p_kv.tile([P, S], f32, tag="uz", bufs=2)
                    nc.vector.tensor_scalar(u_t, ps, 0.5 / S_W,
                                            bias_ap(2, ot),
                                            Alu.mult, Alu.add)
                    t_t = p_kv.tile([P, S], f32, tag="tz", bufs=2)
                    nc.scalar.activation(t_t, ps, AF.Tanh,
                                         bias=bias_ap(2, ot),
                                         scale=0.5 / S_W)
                    nc.vector.scalar_tensor_tensor(
                        z8[:, ot, :], t_t, 1.0, u_t, Alu.add, Alu.mult)

            # K.T -> kt_in (feature-major, partition-contiguous), gather
            # ASAP; half-0 is stored while half-1 computes so the gather
            # kick only waits on the second 0.25MB store
            ktS = p_kv.tile([P, FC, S], fp8)
            kt_dram = kt_in[:].rearrange("(p c s) -> p c s", p=P, s=S)
            proj8("wk", z8, ktS, AF.Identity, 4, 1.0 / S_W,
                  half_done=lambda h: nc.sync.dma_start(
                      out=kt_dram[:, h * 4:(h + 1) * 4, :],
                      in_=ktS[:, h * 4:(h + 1) * 4, :]))
            if not nocoll:
                nc.gpsimd.collective_compute(
                    "AllGather", mybir.AluOpType.bypass,
                    replica_groups=[list(range(NCORES))],
                    ins=[kt_in[:].opt()], outs=[kt_out[:].opt()],
                )
            # next rep's rt / first wa+wd loads: issued after the gather
            # kick so they never delay it, ~50us before phase 1 needs them
            prefetch_cb()

            # V seq-major: V[s, o] = sum_d Z.T[d, s] Wv.T[d, o]; x S_V + bias
            for half in range(2):
                osl = slice(half * 4 * P, (half + 1) * 4 * P)
                wv_t = w8["wv"][half]
                for st in range(4):
                    ssl = slice(st * P, (st + 1) * P)
                    ps = next_ps()[:, 0:4 * P]
                    for cp in range(FC // 2):
                        mm(ps, z8[:, 2 * cp:2 * cp + 2, ssl],
                           wv_t[:, 2 * cp:2 * cp + 2, :],
                           start=(cp == 0), stop=(cp == FC // 2 - 1),
                           perf_mode=DR)
                    v_sb = p_kv.tile([P, 4 * P], fp8, tag="vsb")
                    # v8 = ps*(S_V/S_W) + S_V*b_v  (bv_b pre-scaled on host)
                    nc.vector.scalar_tensor_tensor(
                        v_sb, ps, S_V / S_W, bv_b[:, osl],
                        Alu.mult, Alu.add)
                    nc.sync.dma_start(
                        out=v_in[st * P * D:(st + 1) * P * D].rearrange(
                            "(p o) -> p o", p=P)[:, osl],
                        in_=v_sb)
            if not nocoll:
                nc.gpsimd.collective_compute(
                    "AllGather", mybir.AluOpType.bypass,
                    replica_groups=[list(range(NCORES))],
                    ins=[v_in[:].opt()], outs=[v_out[:].opt()],
                )

            # Q (overlaps the AllGathers)
            proj8("wq", z8, qT8, AF.Identity, 3, 1.0 / S_W)

            # -------- Phase 4: R_EMA' (bf16), f, i (overlap AllGather) ---
            def proj_bf(w_halves, rhs_src, out_tile, func, bias_row):
                for half in range(2):
                    w_t = w_halves[half]
                    for sub in range(4):
                        ot = half * 4 + sub
                        ow = slice(sub * P, (sub + 1) * P)
                        ps = next_ps()
                        for ch in range(FC):
                            mm(ps, w_t[:, ch, ow], rhs_src[:, ch, :],
                               start=(ch == 0), stop=(ch == FC - 1))
                        nc.scalar.activation(out_tile[:, ot, :], ps, func,
                                             bias=bias_ap(bias_row, ot),
                                             scale=1.0)
                        if out_tile is remap:
                            nc.scalar.copy(remap8[:, ot, :],
                                           remap[:, ot, :])

            proj_bf(wema_sb, rema, remap, AF.Identity, 6)
            # f-gate: fT holds tanh(fpre/2); f = 0.5*(1+fT) is folded into
            # the 1/l factor later (ones_dr=2 pre-doubles the denominator).
            # bias row 7 holds b_f/2 (host-packed).
            proj8("wf", remap8, fT, AF.Tanh, 7, 0.5 / S_W)

            wi_sb = p_kv.tile([P, FC], bf16, tag="wi")
            nc.sync.dma_start(out=wi_sb, in_=wi.ap())
            wfin_sb = p_kv.tile([P, FC], bf16, tag="wfin")
            nc.sync.dma_start(out=wfin_sb, in_=wfin.ap())
            ps_i = bank[7][0:1, :]
            for ch in range(FC):
                mm(ps_i, wi_sb[:, ch:ch + 1], rema[:, ch, :],
                   start=(ch == 0), stop=(ch == FC - 1))
            nc.scalar.activation(i_row, ps_i, AF.Tanh,
                                 bias=bsb[0:1, 9, 0:1], scale=1.0)

            # fin_rem = remap @ W_final.T  (the (1-i) branch of the head)
            ps_fr = bank[0][0:1, :]
            for ch in range(FC):
                mm(ps_fr, wfin_sb[:, ch:ch + 1], remap[:, ch, :],
                   start=(ch == 0), stop=(ch == FC - 1))
            nc.scalar.copy(fin_rem, ps_fr)

    # next rep's phase 1: its matmuls fill the PE while this rep's
    # K/V AllGather completes
    emit_next_p1()

    # ---------------- Phase 5+6: attention + head ----------------
    # One PSUM pool of 8 named bank tiles shared by 5A (score chains rotate
    # banks 0-2, denominator lives in a bank-3 row), 5B (zat accumulators in
    # all 8) and the head (chains reuse each bank as its zatp drain
    # completes; the final row-matmul reuses a bank-0 row). Bank-granular
    # tile reuse gives WAR deps instead of pool-alloc barriers, so each
    # stage starts as soon as its first operand is ready.
    with tc.tile_pool(name="p_pt", bufs=1) as p_pt, \
         tc.tile_pool(name="p6t", bufs=1) as p6t, \
         tc.tile_pool(name="p6w", bufs=1) as p6w:
        pt = p_pt.tile([P, NJ, S], fp8)
        l_ps = bank[3][0:1, :]
        wzat_t = [p6w.tile([P, FC, 4 * P], fp8, name=f"wzat{h}")
                  for h in range(2)]
        wfin_sb6 = p6t.tile([P, FC], bf16, tag="wfin6")
        nc.sync.dma_start(out=wfin_sb6, in_=wfin.ap())
        junk = p6t.tile([P, 1], f32, tag="junk")
        # preload the exp table set while waiting on the K.T AllGather
        nc.scalar.activation(junk, bsb[:, 0, 0:1], AF.Exp, bias=0.0,
                             scale=0.0)

        # --- 5A: scores.T + exp; denominator trails one rank behind ---
        def l_pairs(r, start, stop):
            for h in range(2):
                mm(l_ps, ones_dr[:, :, 0:1],
                   pt[:, 4 * r + 2 * h:4 * r + 2 * h + 2, :],
                   start=(start and h == 0), stop=(stop and h == 1),
                   perf_mode=DR)

        with tc.tile_pool(name="p5w", bufs=2) as p5w:
            for r in range(NCORES):
                kt_r = p5w.tile([P, FC, S], fp8, tag="ktr")
                kt_src = kt_in[:] if nocoll else kt_out[r]
                kt_ap = kt_src.rearrange("(p c s) -> p c s", p=P, s=S)
                nc.sync.dma_start(out=kt_r[:, 0:FC // 2, :],
                                  in_=kt_ap[:, 0:FC // 2, :])
                nc.sync.dma_start(out=kt_r[:, FC // 2:, :],
                                  in_=kt_ap[:, FC // 2:, :])
                if r == 4:
                    for h in range(2):
                        nc.sync.dma_start(out=wzat_t[h],
                                          in_=w_in["wzat"].ap()[h])
                for cl in range(4):
                    jc = r * 4 + cl
                    jw = slice(cl * P, (cl + 1) * P)
                    s_ps = bank[(r * 4 + cl) % 3]
                    for cp in range(FC // 2):
                        mm(s_ps, kt_r[:, 2 * cp:2 * cp + 2, jw],
                           qT8[:, 2 * cp:2 * cp + 2, :],
                           start=(cp == 0), stop=(cp == FC // 2 - 1),
                           perf_mode=DR)
                    # pt = S_P * exp(score): ln(S_P) folded into the bias
                    nc.scalar.activation(pt[:, jc, :], s_ps, AF.Exp,
                                         bias=lnsp_col, scale=ATT_SCALE)
                # denominator pairs for the previous rank: their exps are
                # long done, so these never stall the in-order PE queue
                if r >= 1:
                    l_pairs(r - 1, start=(r == 1), stop=False)
            l_pairs(NCORES - 1, start=False, stop=True)

        # preload the tanh set for the head; all exps are already queued
        nc.scalar.activation(junk, bsb[:, 0, 0:1], AF.Tanh, bias=0.0,
                             scale=0.0)

        with tc.tile_pool(name="p5v", bufs=2) as p5v:
            # prefetch rank-0 V before the reciprocal chain so its DMAs
            # are not queued behind the bounce round-trip
            v_r0 = p5v.tile([P, 4, D], fp8, tag="vr")
            v_src0 = v_in[:] if nocoll else v_out[0]
            v_ap0 = v_src0.rearrange("(t p o) -> p t o", p=P, o=D)
            nc.sync.dma_start(out=v_r0[:, 0:2, :], in_=v_ap0[:, 0:2, :])
            nc.sync.dma_start(out=v_r0[:, 2:4, :], in_=v_ap0[:, 2:4, :])


            # l_ps = 2*S_P*l (ones_dr=2), so rl = 0.5/(S_P*l); the 0.5 is
            # the sigmoid-via-tanh constant of the f gate. ~18-bit
            # reciprocal is plenty for a uniform per-column softmax scale.
            l_row = p6t.tile([1, S], f32, tag="lrow")
            nc.vector.reciprocal_approx_fast(l_row, l_ps)
            nc.sync.dma_start(out=row_bounce[0:1, :], in_=l_row)
            rl_b = p6t.tile([P, S], f32, tag="rlb")
            nc.sync.dma_start(out=rl_b,
                              in_=row_bounce[0:1, :].partition_broadcast(P))
            # fT <- (tanh(fpre/2) + 1) * 0.5/(S_P*l)  ==  f/(S_P*l)
            for ot in range(FC):
                nc.vector.scalar_tensor_tensor(
                    fT[:, ot, :], fT[:, ot, :], 1.0, rl_b,
                    Alu.add, Alu.mult)

            # --- 5B: Zat.T (whole-rank V loads) ---
            for r in range(NCORES):
                if r == 0:
                    v_r = v_r0
                else:
                    v_r = p5v.tile([P, 4, D], fp8, tag="vr")
                    v_src = v_in[:] if nocoll else v_out[r]
                    v_ap = v_src.rearrange("(t p o) -> p t o", p=P, o=D)
                    nc.sync.dma_start(out=v_r[:, 0:2, :],
                                      in_=v_ap[:, 0:2, :])
                    nc.sync.dma_start(out=v_r[:, 2:4, :],
                                      in_=v_ap[:, 2:4, :])
                for u in range(2):
                    jp = r * 2 + u
                    # first group: bank 3 last, so its WAR on the
                    # denominator-reciprocal read never stalls the queue
                    ots = [0, 1, 2, 4, 5, 6, 7, 3] if jp == 0 else range(FC)
                    for ot in ots:
                        mm(bank[ot],
                           v_r[:, 2 * u:2 * u + 2, ot * P:(ot + 1) * P],
                           pt[:, 4 * r + 2 * u:4 * r + 2 * u + 2, :],
                           start=(jp == 0), stop=(jp == NJ // 2 - 1),
                           perf_mode=DR)

        # ---------------- Phase 6: output head ----------------
        # p_hat = sigmoid(i*(Ztp@Wf) + (1-i)*(remap@Wf)); remap@Wf in ph 4.
        # zatp drain, head chain and tanh pipeline per-bank: chain ot
        # starts as soon as bank ot is drained, while later banks finish.
        ztps = []
        for ot in range(FC):
            nc.vector.tensor_mul(zatp8[:, ot, :], bank[ot], fT[:, ot, :])
        for half in range(2):
            w_t = wzat_t[half]
            for sub in range(4):
                ot = half * 4 + sub
                ow = slice(sub * P, (sub + 1) * P)
                ps = bank[ot]
                for cp in range(FC // 2):
                    mm(ps, w_t[:, 2 * cp:2 * cp + 2, ow],
                       zatp8[:, 2 * cp:2 * cp + 2, :],
                       start=(cp == 0), stop=(cp == FC // 2 - 1),
                       perf_mode=DR)
                t_sum = p6t.tile([P, S], f32, tag="tsum", bufs=2)
                # t_sum = ps/(S_V*S_W) + remap
                nc.vector.scalar_tensor_tensor(
                    t_sum, ps, 1.0 / (S_V * S_W), remap[:, ot, :],
                    Alu.mult, Alu.add)
                ztp = p6t.tile([P, S], bf16, tag=f"ztp{ot}",
                               name=f"ztp{ot}")
                nc.scalar.activation(ztp, t_sum, AF.Tanh,
                                     bias=bias_ap(8, ot), scale=1.0)
                ztps.append(ztp)
        # fin matmuls after all tanh tiles: no per-tile PE stall on ACT.
        # The [1,S] accumulator reuses a bank-0 row (bank 0 is long drained).
        fin_ps = bank[0][0:1, :]
        for ot in range(FC):
            mm(fin_ps, wfin_sb6[:, ot:ot + 1], ztps[ot],
               start=(ot == 0), stop=(ot == FC - 1))
        # blend the two head branches on the [1, S] rows
        d_r = p6t.tile([1, S], f32, tag="dr")
        nc.vector.tensor_sub(d_r, fin_ps, fin_rem)
        m_r = p6t.tile([1, S], f32, tag="mr")
        nc.vector.tensor_mul(m_r, d_r, i_row)
        s_r = p6t.tile([1, S], f32, tag="sr")
        nc.vector.tensor_add(s_r, m_r, fin_rem)
        # sigmoid via tanh: p = 0.5*(1 + tanh(x/2)) — stays in the tanh set
        t_fin = p6t.tile([1, S], f32, tag="tfin")
        nc.scalar.activation(t_fin, s_r, AF.Tanh, bias=0.0, scale=0.5)
        phat = p6t.tile([1, S], f32, tag="phat")
        nc.vector.tensor_scalar(phat, t_fin, 0.5, 0.5, Alu.mult, Alu.add)
        nc.sync.dma_start(out=out.ap().rearrange("s o -> o s"), in_=phat)


def _prep_host_inputs(inputs):
    """Transpose/pack weights into SBUF layouts + build per-core shards."""
    import concourse.mybir as mybir
    bf16 = mybir.dt.np(mybir.dt.bfloat16)
    fp8 = mybir.dt.np(mybir.dt.float8e4)

    R = np.ascontiguousarray(inputs["R"], dtype=np.float32)
    RT_ext = np.concatenate(
        [np.zeros((D, 1), np.float32), np.ascontiguousarray(R.T)], axis=1)

    def pack_p1(w):  # W [D, 2D] -> W.T [2D, D] -> [ot, p, k(2FC), o(P)]
        wt = np.ascontiguousarray(np.asarray(w, np.float32).T)
        return np.ascontiguousarray(
            wt.reshape(2 * FC, P, FC, P).transpose(2, 1, 0, 3)).astype(bf16)

    def pack_proj(w, dt, scale=1.0):
        # W [D, D] -> W.T [D, D] -> [half, p, c(FC), o(4P)]
        wt = np.ascontiguousarray(np.asarray(w, np.float32).T * scale)
        return np.ascontiguousarray(
            wt.reshape(FC, P, 2, 4 * P).transpose(2, 1, 0, 3)).astype(dt)

    def pack_col(w):  # W [1, D] -> [p, c(FC)]
        wt = np.asarray(w, np.float32).reshape(FC, P).T
        return np.ascontiguousarray(wt).astype(bf16)

    w = {
        "wa": pack_p1(inputs["W_alpha"]), "wd": pack_p1(inputs["W_delta"]),
        "wz": pack_proj(inputs["W_z"], fp8, S_W),
        "wq": pack_proj(inputs["W_q"], fp8, S_W),
        "wk": pack_proj(inputs["W_k"], fp8, S_W),
        "wv": pack_proj(inputs["W_v"], fp8, S_W),
        "wf": pack_proj(inputs["W_f"], fp8, S_W),
        "wzat": pack_proj(inputs["W_z_at"], fp8, S_W),
        "wema": pack_proj(inputs["W_EMA"], bf16),
        "wi": pack_col(inputs["W_i"]), "wfin": pack_col(inputs["W_final"]),
    }

    brows = np.zeros((10, D), np.float32)
    brows[0] = inputs["b_alpha"]
    brows[1] = inputs["b_delta"]
    brows[2] = np.asarray(inputs["b_z"]) * 0.5   # silu-via-tanh uses b_z/2
    brows[3] = inputs["b_q"]
    brows[4] = inputs["b_k"]
    brows[5] = inputs["b_v"]
    brows[6] = inputs["b_EMA"]
    brows[7] = np.asarray(inputs["b_f"]) * 0.5   # sigmoid-via-tanh: b_f/2
    brows[8] = inputs["b_z_at"]
    brows[9, 0] = np.float32(inputs["b_i"][0])
    biases = np.ascontiguousarray(
        brows.reshape(10, FC, P).transpose(2, 0, 1))  # [p, 10, FC]
    bvrow = np.ascontiguousarray(brows[5:6] * S_V)  # [1, D], pre-scaled

    in_maps = []
    for c in range(NCORES):
        rt_c = RT_ext[:, c * S:c * S + S + 1]  # [D, S+1]
        rt_p = np.ascontiguousarray(
            rt_c.reshape(FC, P, S + 1).transpose(1, 0, 2)).astype(bf16)
        m = {"rt": rt_p, "biases": biases, "bvrow": bvrow}
        m.update(w)
        in_maps.append(m)
    return in_maps


def kernel(**inputs):
    from concourse.bass_utils import run_bass_kernel_spmd

    if "nc" not in _CACHE:
        _CACHE["nc"] = _build_bass()
    nc = _CACHE["nc"]
    in_maps = _prep_host_inputs(inputs)
    res = run_bass_kernel_spmd(nc, in_maps, core_ids=list(range(NCORES)))
    outs = [res.results[c]["out"] for c in range(NCORES)]
    return np.concatenate(outs, axis=0).astype(np.float32)
